# revision 65
# baseline (speedup 1.0000x reference)
"""Trainium2 Bass kernel for nn_Policy (40-layer 32-wide MLP over B=131072).

Strategy (pure data parallel over 8 cores, 16384 rows/core):
  - Activations live TRANSPOSED in SBUF: [32 features x batch] replicated in
    4 partition strips (strip t = partitions 32t..32t+32), 8 x 512-col slices
    per strip -> 32 chunks of 512 batch rows per core.
  - Each hidden layer = 2 rounds of 16 concurrent 32x32 tile-packed matmuls
    (tile_position=(32r, 32c)): weights stationary, activations moving.
    Tile (r, c) reads strip r slice 4k+c, writes PSUM bank r partitions 32c.
  - Fused bias+ReLU PSUM->SBUF, split between ScalarE (activation Relu with
    per-partition bias) and VectorE (dual-op tensor_scalar add/max).
  - Input transpose: contiguous DMA loads then PE transpose-mode 128x128.
  - Final layer: per chunk a [32,32] stationary with W_out in column s so 8
    chunks accumulate into distinct rows of one PSUM bank; one ScalarE copy
    (+b_out) then strided DMA out.
"""

import numpy as np

BC = 16384  # batch rows per core
D = 32
L = 40
NCORES = 8
NSTRIP = 4   # partition strips
NSLICE = 8   # 512-wide slices per strip
CHUNK = 512
SPLIT = 1104  # free-dim split point: ScalarE does [0:SPLIT], VectorE the rest
USE_F32R = True  # single-pass fp32 matmuls (4x PE throughput, ~tf32 precision)
LDW_OPT = True  # let walrus dedupe identical consecutive LDWEIGHTS

_CACHE = {}


def _patch_ldw_opt():
    if not LDW_OPT or _CACHE.get("ldw_patched"):
        return
    import concourse.bass_utils as bu

    orig = bu.run_command

    def patched(argv, **kw):
        argv = [
            "--enable-ldw-opt=true" if a == "--enable-ldw-opt=false" else a
            for a in argv
        ]
        return orig(argv, **kw)

    bu.run_command = patched
    _CACHE["ldw_patched"] = True


def _build_nc():
    import concourse.bacc as bacc
    import concourse.tile as tile
    from concourse import mybir

    _patch_ldw_opt()

    f32 = mybir.dt.float32
    fmm = mybir.dt.float32r if USE_F32R else f32
    nc = bacc.Bacc(
        "TRN2", target_bir_lowering=False, debug=False, enable_asserts=False
    )

    x_d = nc.dram_tensor("x", [BC, D], fmm, kind="ExternalInput").ap()
    w4_d = nc.dram_tensor("w4", [128, L * 32], fmm, kind="ExternalInput").ap()
    b4_d = nc.dram_tensor("b4", [128, L], f32, kind="ExternalInput").ap()
    wfin_d = nc.dram_tensor("wfin", [128, 1024], fmm, kind="ExternalInput").ap()
    bout_d = nc.dram_tensor("bout", [128, 1], f32, kind="ExternalInput").ap()
    ident_d = nc.dram_tensor("ident", [128, 128], fmm, kind="ExternalInput").ap()
    y_d = nc.dram_tensor("y", [BC, 1], f32, kind="ExternalOutput").ap()

    def mmcast(ap):
        return ap

    with tile.TileContext(nc) as tc:
        with (
            tc.tile_pool(name="consts", bufs=1) as consts,
            tc.tile_pool(name="xin", bufs=1) as xinp,
            tc.tile_pool(name="acts", bufs=3) as actp,
            tc.tile_pool(name="fin", bufs=1) as finp,
            tc.tile_pool(name="psum", bufs=4, space="PSUM") as psump,
        ):
            w4_sb = consts.tile([128, L * 128], fmm, tag="w4")
            b4_sb = consts.tile([128, L], f32, tag="b4")
            wfin_sb = consts.tile([128, 1024], fmm, tag="wfin")
            bout_sb = consts.tile([128, 1], f32, tag="bout")
            ident_sb = consts.tile([128, 128], fmm, tag="ident")
            nc.gpsimd.memset(w4_sb.bitcast(mybir.dt.uint32), 0)
            nc.sync.dma_start(out=ident_sb, in_=ident_d)

            # Input staging, fully contiguous: xin[p, 32u + i] = x[128p + u, i]
            # (16KB contiguous per partition -> full DMA bandwidth). The
            # resulting batch permutation is undone in the output DMA.
            xin_sb = xinp.tile([128, BC // 4], fmm, tag="xin")
            x_c = x_d.rearrange("(p u) i -> p (u i)", p=128)
            for g in range(8):
                eng = nc.sync if g % 2 == 0 else nc.scalar
                eng.dma_start(
                    out=xin_sb[:, 512 * g : 512 * (g + 1)],
                    in_=x_c[:, 512 * g : 512 * (g + 1)],
                )

            # Block-diagonal weights: zero-fill (above) then 4 strided DMAs
            # of the compact [128, L*32] replicated weights into the blocks.
            w4bd = w4_sb.rearrange("p (l m) -> p l m", m=128)
            for r in range(NSTRIP):
                nc.scalar.dma_start(
                    out=w4bd[32 * r : 32 * r + 32, :, 32 * r : 32 * r + 32],
                    in_=w4_d[32 * r : 32 * r + 32, :].rearrange(
                        "i (l o) -> i l o", o=32
                    ),
                )
            nc.gpsimd.dma_start(out=b4_sb, in_=b4_d)
            nc.gpsimd.dma_start(out=bout_sb, in_=bout_d)
            nc.gpsimd.dma_start(out=wfin_sb, in_=wfin_d)

            # Transpose into strip layout:
            # act[32t + i, 128n + p] = x[512n + 128t + p, i]
            act = actp.tile([128, BC // 4], fmm, tag="act")
            for g in range(4):
                ps = psump.tile([128, 1024], fmm, tag="ps")
                for j in range(8):
                    n = 8 * g + j
                    nc.tensor.transpose(
                        ps[:, 128 * j : 128 * j + 128],
                        xin_sb[:, 128 * n : 128 * n + 128],
                        ident_sb,
                    )
                base = 1024 * g
                if g % 2 == 0:
                    nc.scalar.activation(
                        act[:, base : base + 1024],
                        ps[:, 0:1024],
                        mybir.ActivationFunctionType.Copy,
                    )
                else:
                    nc.vector.tensor_copy(act[:, base : base + 1024], ps[:, 0:1024])

            # Hidden layers
            prev = act
            for l in range(L):
                cur = actp.tile([128, BC // 4], fmm, tag="act")
                for u in range(4):
                    ps = psump.tile([128, 1024], f32, tag="ps")
                    for j in range(2):
                        # blockdiag(W_l.T x4): one MM does 4 strips' chunks
                        col = 1024 * u + 512 * j
                        nc.tensor.matmul(
                            ps[:, 512 * j : 512 * j + 512],
                            w4_sb[:, 128 * l : 128 * l + 128],
                            prev[:, col : col + 512],
                            start=True,
                            stop=True,
                        )
                    base = 1024 * u
                    if u % 2 == 1:
                        nc.scalar.activation(
                            cur[:, base : base + 1024],
                            ps[:, 0:1024],
                            mybir.ActivationFunctionType.Relu,
                            bias=b4_sb[:, l : l + 1],
                        )
                    else:
                        nc.vector.tensor_scalar(
                            cur[:, base : base + 1024],
                            ps[:, 0:1024],
                            b4_sb[:, l : l + 1],
                            0.0,
                            mybir.AluOpType.add,
                            mybir.AluOpType.max,
                        )
                prev = cur

            # Final layer, blockdiag: MM s writes y of chunk (t, s) to
            # psum partition 32t + s; 8 accumulating MMs share one bank.
            fin_ps = psump.tile([128, 1024], f32, tag="ps", name="fin_ps")
            for s in range(NSLICE):
                nc.tensor.matmul(
                    fin_ps[:, 0:512],
                    wfin_sb[:, 128 * s : 128 * s + 128],
                    prev[:, 512 * s : 512 * s + 512],
                    start=(s == 0),
                    stop=(s == NSLICE - 1),
                )
            # fin_ps[32t + s, 128c + p] = y[128p + 16s + 4c + t] (no bias yet).
            # Copy to SBUF, PE-transpose back to batch-major, add bias with a
            # shuffled output AP, then one contiguous DMA out.
            fin_sb = finp.tile([128, 512], f32, tag="fin")
            nc.scalar.activation(
                fin_sb, fin_ps[:, 0:512], mybir.ActivationFunctionType.Copy
            )
            tr_ps = psump.tile([128, 1024], f32, tag="ps", name="tr_ps")
            for c in range(4):
                nc.tensor.transpose(
                    tr_ps[:, 128 * c : 128 * c + 128],
                    fin_sb[:, 128 * c : 128 * c + 128],
                    ident_sb.bitcast(f32),
                )
            # tr_ps[p, 128c + 32t + s] -> fin2[p, 16s + 4c + t]
            fin2 = finp.tile([128, 128], f32, tag="fin2")
            nc.scalar.activation(
                fin2.rearrange("p (s c t) -> p c t s", s=8, c=4, t=4),
                tr_ps[:, 0:512].rearrange("p (c t s) -> p c t s", c=4, t=4, s=32)[
                    :, :, :, 0:8
                ],
                mybir.ActivationFunctionType.Identity,
                bias=bout_sb,
            )
            nc.sync.dma_start(
                out=y_d.rearrange("(p q) o -> p (q o)", p=128), in_=fin2
            )

    nc.compile()
    return nc


def _prep_weights(W_hidden, b_hidden, W_out, b_out):
    W_hidden = np.asarray(W_hidden, dtype=np.float32)
    b_hidden = np.asarray(b_hidden, dtype=np.float32)
    W_out = np.asarray(W_out, dtype=np.float32)
    b_out = np.asarray(b_out, dtype=np.float32)

    # w4[32r + i, 32l + o] = W_hidden[l, o, i]  (compact WT, replicated x4;
    # the kernel zero-fills and scatters this into the block-diagonal form)
    w4 = np.tile(W_hidden.transpose(2, 0, 1).reshape(32, L * 32), (4, 1)).copy()
    # b4[32c + o, l] = b_hidden[l, o]
    b4 = np.tile(b_hidden.T, (4, 1)).copy()
    # wfin[32t + i, 128s + m] = W_out[0, i] iff m == 32t + s  (s = 0..7)
    wfin = np.zeros((128, 1024), dtype=np.float32)
    for t in range(NSTRIP):
        for s in range(NSLICE):
            wfin[32 * t : 32 * t + 32, 128 * s + 32 * t + s] = W_out[0, :]
    bout = np.full((128, 1), b_out[0], dtype=np.float32)
    ident = np.eye(128, dtype=np.float32)
    return w4, b4, wfin, bout, ident


def kernel(state, W_hidden, b_hidden, W_out, b_out):
    from concourse import bass_utils

    state = np.asarray(state, dtype=np.float32)
    w4, b4, wfin, bout, ident = _prep_weights(W_hidden, b_hidden, W_out, b_out)

    if "nc" not in _CACHE:
        _CACHE["nc"] = _build_nc()
    nc = _CACHE["nc"]

    in_maps = []
    for i in range(NCORES):
        in_maps.append(
            {
                "x": state[BC * i : BC * (i + 1)].copy(),
                "w4": w4,
                "b4": b4,
                "wfin": wfin,
                "bout": bout,
                "ident": ident,
            }
        )
    res = bass_utils.run_bass_kernel_spmd(nc, in_maps, core_ids=list(range(NCORES)))
    y = np.concatenate([res.results[i]["y"] for i in range(NCORES)], axis=0)
    return y.astype(np.float32)


# revision 66
# speedup vs baseline: 1.0054x; 1.0054x over previous
"""Trainium2 Bass kernel for nn_Policy (40-layer 32-wide MLP over B=131072).

Strategy (pure data parallel over 8 cores, 16384 rows/core):
  - Activations live TRANSPOSED in SBUF: [32 features x batch] replicated in
    4 partition strips (strip t = partitions 32t..32t+32), 8 x 512-col slices
    per strip -> 32 chunks of 512 batch rows per core.
  - Each hidden layer = 2 rounds of 16 concurrent 32x32 tile-packed matmuls
    (tile_position=(32r, 32c)): weights stationary, activations moving.
    Tile (r, c) reads strip r slice 4k+c, writes PSUM bank r partitions 32c.
  - Fused bias+ReLU PSUM->SBUF, split between ScalarE (activation Relu with
    per-partition bias) and VectorE (dual-op tensor_scalar add/max).
  - Input transpose: contiguous DMA loads then PE transpose-mode 128x128.
  - Final layer: per chunk a [32,32] stationary with W_out in column s so 8
    chunks accumulate into distinct rows of one PSUM bank; one ScalarE copy
    (+b_out) then strided DMA out.
"""

import numpy as np

BC = 16384  # batch rows per core
D = 32
L = 40
NCORES = 8
NSTRIP = 4   # partition strips
NSLICE = 8   # 512-wide slices per strip
CHUNK = 512
SPLIT = 1104  # free-dim split point: ScalarE does [0:SPLIT], VectorE the rest
USE_F32R = True  # single-pass fp32 matmuls (4x PE throughput, ~tf32 precision)
LDW_OPT = True  # let walrus dedupe identical consecutive LDWEIGHTS

_CACHE = {}


def _patch_ldw_opt():
    if not LDW_OPT or _CACHE.get("ldw_patched"):
        return
    import concourse.bass_utils as bu

    orig = bu.run_command

    def patched(argv, **kw):
        argv = [
            "--enable-ldw-opt=true" if a == "--enable-ldw-opt=false" else a
            for a in argv
        ]
        return orig(argv, **kw)

    bu.run_command = patched
    _CACHE["ldw_patched"] = True


def _build_nc():
    import concourse.bacc as bacc
    import concourse.tile as tile
    from concourse import mybir

    _patch_ldw_opt()

    f32 = mybir.dt.float32
    fmm = mybir.dt.float32r if USE_F32R else f32
    nc = bacc.Bacc(
        "TRN2", target_bir_lowering=False, debug=False, enable_asserts=False
    )

    x_d = nc.dram_tensor("x", [BC, D], fmm, kind="ExternalInput").ap()
    w4_d = nc.dram_tensor("w4", [128, L * 32], fmm, kind="ExternalInput").ap()
    b4_d = nc.dram_tensor("b4", [128, L], f32, kind="ExternalInput").ap()
    wfin_d = nc.dram_tensor("wfin", [128, 1024], fmm, kind="ExternalInput").ap()
    bout_d = nc.dram_tensor("bout", [128, 1], f32, kind="ExternalInput").ap()
    ident_d = nc.dram_tensor("ident", [128, 128], fmm, kind="ExternalInput").ap()
    y_d = nc.dram_tensor("y", [BC, 1], f32, kind="ExternalOutput").ap()

    def mmcast(ap):
        return ap

    with tile.TileContext(nc) as tc:
        with (
            tc.tile_pool(name="consts", bufs=1) as consts,
            tc.tile_pool(name="xin", bufs=1) as xinp,
            tc.tile_pool(name="acts", bufs=3) as actp,
            tc.tile_pool(name="fin", bufs=1) as finp,
            tc.tile_pool(name="psum", bufs=4, space="PSUM") as psump,
        ):
            w4_sb = consts.tile([128, L * 128], fmm, tag="w4")
            b4_sb = consts.tile([128, L], f32, tag="b4")
            wfin_sb = consts.tile([128, 1024], fmm, tag="wfin")
            bout_sb = consts.tile([128, 1], f32, tag="bout")
            ident_sb = consts.tile([128, 128], fmm, tag="ident")
            nc.gpsimd.memset(w4_sb.bitcast(mybir.dt.uint32), 0)
            nc.sync.dma_start(out=ident_sb, in_=ident_d)

            # Input staging, fully contiguous: xin[p, 32u + i] = x[128p + u, i]
            # (16KB contiguous per partition -> full DMA bandwidth). The
            # resulting batch permutation is undone in the output DMA.
            xin_sb = xinp.tile([128, BC // 4], fmm, tag="xin")
            x_c = x_d.rearrange("(p u) i -> p (u i)", p=128)
            for g in range(8):
                eng = nc.sync if g % 2 == 0 else nc.scalar
                eng.dma_start(
                    out=xin_sb[:, 512 * g : 512 * (g + 1)],
                    in_=x_c[:, 512 * g : 512 * (g + 1)],
                )

            # Block-diagonal weights: zero-fill (above) then 4 strided DMAs
            # of the compact [128, L*32] replicated weights into the blocks.
            w4bd = w4_sb.rearrange("p (l m) -> p l m", m=128)
            for r in range(NSTRIP):
                nc.scalar.dma_start(
                    out=w4bd[32 * r : 32 * r + 32, :, 32 * r : 32 * r + 32],
                    in_=w4_d[32 * r : 32 * r + 32, :].rearrange(
                        "i (l o) -> i l o", o=32
                    ),
                )
            nc.gpsimd.dma_start(out=b4_sb, in_=b4_d)
            nc.gpsimd.dma_start(out=bout_sb, in_=bout_d)
            nc.gpsimd.dma_start(out=wfin_sb, in_=wfin_d)

            # Transpose into strip layout:
            # act[32t + i, 128n + p] = x[512n + 128t + p, i]
            act = actp.tile([128, BC // 4], fmm, tag="act")
            for g in range(4):
                ps = psump.tile([128, 1024], fmm, tag="ps")
                for j in range(8):
                    n = 8 * g + j
                    nc.tensor.transpose(
                        ps[:, 128 * j : 128 * j + 128],
                        xin_sb[:, 128 * n : 128 * n + 128],
                        ident_sb,
                    )
                base = 1024 * g
                if g % 2 == 0:
                    nc.scalar.activation(
                        act[:, base : base + 1024],
                        ps[:, 0:1024],
                        mybir.ActivationFunctionType.Copy,
                    )
                else:
                    nc.vector.tensor_copy(act[:, base : base + 1024], ps[:, 0:1024])

            # Hidden layers
            prev = act
            for l in range(L):
                cur = actp.tile([128, BC // 4], fmm, tag="act")
                for u in range(4):
                    ps = psump.tile([128, 1024], f32, tag="ps")
                    for j in range(2):
                        # blockdiag(W_l.T x4): one MM does 4 strips' chunks
                        col = 1024 * u + 512 * j
                        nc.tensor.matmul(
                            ps[:, 512 * j : 512 * j + 512],
                            w4_sb[:, 128 * l : 128 * l + 128],
                            prev[:, col : col + 512],
                            start=True,
                            stop=True,
                        )
                    base = 1024 * u
                    if u % 2 == 0:
                        nc.scalar.activation(
                            cur[:, base : base + 1024],
                            ps[:, 0:1024],
                            mybir.ActivationFunctionType.Relu,
                            bias=b4_sb[:, l : l + 1],
                        )
                    else:
                        nc.vector.tensor_scalar(
                            cur[:, base : base + 1024],
                            ps[:, 0:1024],
                            b4_sb[:, l : l + 1],
                            0.0,
                            mybir.AluOpType.add,
                            mybir.AluOpType.max,
                        )
                prev = cur

            # Final layer, blockdiag: MM s writes y of chunk (t, s) to
            # psum partition 32t + s; 8 accumulating MMs share one bank.
            fin_ps = psump.tile([128, 1024], f32, tag="ps", name="fin_ps")
            for s in range(NSLICE):
                nc.tensor.matmul(
                    fin_ps[:, 0:512],
                    wfin_sb[:, 128 * s : 128 * s + 128],
                    prev[:, 512 * s : 512 * s + 512],
                    start=(s == 0),
                    stop=(s == NSLICE - 1),
                )
            # fin_ps[32t + s, 128c + p] = y[128p + 16s + 4c + t] (no bias yet).
            # Copy to SBUF, PE-transpose back to batch-major, add bias with a
            # shuffled output AP, then one contiguous DMA out.
            fin_sb = finp.tile([128, 512], f32, tag="fin")
            nc.scalar.activation(
                fin_sb, fin_ps[:, 0:512], mybir.ActivationFunctionType.Copy
            )
            tr_ps = psump.tile([128, 1024], f32, tag="ps", name="tr_ps")
            for c in range(4):
                nc.tensor.transpose(
                    tr_ps[:, 128 * c : 128 * c + 128],
                    fin_sb[:, 128 * c : 128 * c + 128],
                    ident_sb.bitcast(f32),
                )
            # tr_ps[p, 128c + 32t + s] -> fin2[p, 16s + 4c + t]
            fin2 = finp.tile([128, 128], f32, tag="fin2")
            nc.scalar.activation(
                fin2.rearrange("p (s c t) -> p c t s", s=8, c=4, t=4),
                tr_ps[:, 0:512].rearrange("p (c t s) -> p c t s", c=4, t=4, s=32)[
                    :, :, :, 0:8
                ],
                mybir.ActivationFunctionType.Identity,
                bias=bout_sb,
            )
            nc.sync.dma_start(
                out=y_d.rearrange("(p q) o -> p (q o)", p=128), in_=fin2
            )

    nc.compile()
    return nc


def _prep_weights(W_hidden, b_hidden, W_out, b_out):
    W_hidden = np.asarray(W_hidden, dtype=np.float32)
    b_hidden = np.asarray(b_hidden, dtype=np.float32)
    W_out = np.asarray(W_out, dtype=np.float32)
    b_out = np.asarray(b_out, dtype=np.float32)

    # w4[32r + i, 32l + o] = W_hidden[l, o, i]  (compact WT, replicated x4;
    # the kernel zero-fills and scatters this into the block-diagonal form)
    w4 = np.tile(W_hidden.transpose(2, 0, 1).reshape(32, L * 32), (4, 1)).copy()
    # b4[32c + o, l] = b_hidden[l, o]
    b4 = np.tile(b_hidden.T, (4, 1)).copy()
    # wfin[32t + i, 128s + m] = W_out[0, i] iff m == 32t + s  (s = 0..7)
    wfin = np.zeros((128, 1024), dtype=np.float32)
    for t in range(NSTRIP):
        for s in range(NSLICE):
            wfin[32 * t : 32 * t + 32, 128 * s + 32 * t + s] = W_out[0, :]
    bout = np.full((128, 1), b_out[0], dtype=np.float32)
    ident = np.eye(128, dtype=np.float32)
    return w4, b4, wfin, bout, ident


def kernel(state, W_hidden, b_hidden, W_out, b_out):
    from concourse import bass_utils

    state = np.asarray(state, dtype=np.float32)
    w4, b4, wfin, bout, ident = _prep_weights(W_hidden, b_hidden, W_out, b_out)

    if "nc" not in _CACHE:
        _CACHE["nc"] = _build_nc()
    nc = _CACHE["nc"]

    in_maps = []
    for i in range(NCORES):
        in_maps.append(
            {
                "x": state[BC * i : BC * (i + 1)].copy(),
                "w4": w4,
                "b4": b4,
                "wfin": wfin,
                "bout": bout,
                "ident": ident,
            }
        )
    res = bass_utils.run_bass_kernel_spmd(nc, in_maps, core_ids=list(range(NCORES)))
    y = np.concatenate([res.results[i]["y"] for i in range(NCORES)], axis=0)
    return y.astype(np.float32)


# revision 67
# speedup vs baseline: 1.0067x; 1.0013x over previous
"""Trainium2 Bass kernel for nn_Policy (40-layer 32-wide MLP over B=131072).

Strategy (pure data parallel over 8 cores, 16384 rows/core):
  - Activations live TRANSPOSED in SBUF: [32 features x batch] replicated in
    4 partition strips (strip t = partitions 32t..32t+32), 8 x 512-col slices
    per strip -> 32 chunks of 512 batch rows per core.
  - Each hidden layer = 2 rounds of 16 concurrent 32x32 tile-packed matmuls
    (tile_position=(32r, 32c)): weights stationary, activations moving.
    Tile (r, c) reads strip r slice 4k+c, writes PSUM bank r partitions 32c.
  - Fused bias+ReLU PSUM->SBUF, split between ScalarE (activation Relu with
    per-partition bias) and VectorE (dual-op tensor_scalar add/max).
  - Input transpose: contiguous DMA loads then PE transpose-mode 128x128.
  - Final layer: per chunk a [32,32] stationary with W_out in column s so 8
    chunks accumulate into distinct rows of one PSUM bank; one ScalarE copy
    (+b_out) then strided DMA out.
"""

import numpy as np

BC = 16384  # batch rows per core
D = 32
L = 40
NCORES = 8
NSTRIP = 4   # partition strips
NSLICE = 8   # 512-wide slices per strip
CHUNK = 512
SPLIT = 1104  # free-dim split point: ScalarE does [0:SPLIT], VectorE the rest
USE_F32R = True  # single-pass fp32 matmuls (4x PE throughput, ~tf32 precision)
LDW_OPT = True  # let walrus dedupe identical consecutive LDWEIGHTS

_CACHE = {}


def _patch_ldw_opt():
    if not LDW_OPT or _CACHE.get("ldw_patched"):
        return
    import concourse.bass_utils as bu

    orig = bu.run_command

    def patched(argv, **kw):
        argv = [
            "--enable-ldw-opt=true" if a == "--enable-ldw-opt=false" else a
            for a in argv
        ]
        return orig(argv, **kw)

    bu.run_command = patched
    _CACHE["ldw_patched"] = True


def _build_nc():
    import concourse.bacc as bacc
    import concourse.tile as tile
    from concourse import mybir

    _patch_ldw_opt()

    f32 = mybir.dt.float32
    fmm = mybir.dt.float32r if USE_F32R else f32
    nc = bacc.Bacc(
        "TRN2", target_bir_lowering=False, debug=False, enable_asserts=False
    )

    x_d = nc.dram_tensor("x", [BC, D], fmm, kind="ExternalInput").ap()
    w4_d = nc.dram_tensor("w4", [128, L * 32], fmm, kind="ExternalInput").ap()
    b4_d = nc.dram_tensor("b4", [128, L], f32, kind="ExternalInput").ap()
    wfin_d = nc.dram_tensor("wfin", [128, 1024], fmm, kind="ExternalInput").ap()
    bout_d = nc.dram_tensor("bout", [128, 1], f32, kind="ExternalInput").ap()
    ident_d = nc.dram_tensor("ident", [128, 128], fmm, kind="ExternalInput").ap()
    y_d = nc.dram_tensor("y", [BC, 1], f32, kind="ExternalOutput").ap()

    def mmcast(ap):
        return ap

    with tile.TileContext(nc) as tc:
        with (
            tc.tile_pool(name="consts", bufs=1) as consts,
            tc.tile_pool(name="xin", bufs=1) as xinp,
            tc.tile_pool(name="acts", bufs=4) as actp,
            tc.tile_pool(name="fin", bufs=1) as finp,
            tc.tile_pool(name="psum", bufs=4, space="PSUM") as psump,
        ):
            w4_sb = consts.tile([128, L * 128], fmm, tag="w4")
            b4_sb = consts.tile([128, L], f32, tag="b4")
            wfin_sb = consts.tile([128, 1024], fmm, tag="wfin")
            bout_sb = consts.tile([128, 1], f32, tag="bout")
            ident_sb = consts.tile([128, 128], fmm, tag="ident")
            nc.gpsimd.memset(w4_sb.bitcast(mybir.dt.uint32), 0)
            nc.sync.dma_start(out=ident_sb, in_=ident_d)

            # Input staging, fully contiguous: xin[p, 32u + i] = x[128p + u, i]
            # (16KB contiguous per partition -> full DMA bandwidth). The
            # resulting batch permutation is undone in the output DMA.
            xin_sb = xinp.tile([128, BC // 4], fmm, tag="xin")
            x_c = x_d.rearrange("(p u) i -> p (u i)", p=128)
            for g in range(8):
                eng = nc.sync if g % 2 == 0 else nc.scalar
                eng.dma_start(
                    out=xin_sb[:, 512 * g : 512 * (g + 1)],
                    in_=x_c[:, 512 * g : 512 * (g + 1)],
                )

            # Block-diagonal weights: zero-fill (above) then 4 strided DMAs
            # of the compact [128, L*32] replicated weights into the blocks.
            w4bd = w4_sb.rearrange("p (l m) -> p l m", m=128)
            for r in range(NSTRIP):
                nc.scalar.dma_start(
                    out=w4bd[32 * r : 32 * r + 32, :, 32 * r : 32 * r + 32],
                    in_=w4_d[32 * r : 32 * r + 32, :].rearrange(
                        "i (l o) -> i l o", o=32
                    ),
                )
            nc.gpsimd.dma_start(out=b4_sb, in_=b4_d)
            nc.gpsimd.dma_start(out=bout_sb, in_=bout_d)
            nc.gpsimd.dma_start(out=wfin_sb, in_=wfin_d)

            # Transpose into strip layout:
            # act[32t + i, 128n + p] = x[512n + 128t + p, i]
            act = actp.tile([128, BC // 4], fmm, tag="act")
            for g in range(4):
                ps = psump.tile([128, 1024], fmm, tag="ps")
                for j in range(8):
                    n = 8 * g + j
                    nc.tensor.transpose(
                        ps[:, 128 * j : 128 * j + 128],
                        xin_sb[:, 128 * n : 128 * n + 128],
                        ident_sb,
                    )
                base = 1024 * g
                if g % 2 == 0:
                    nc.scalar.activation(
                        act[:, base : base + 1024],
                        ps[:, 0:1024],
                        mybir.ActivationFunctionType.Copy,
                    )
                else:
                    nc.vector.tensor_copy(act[:, base : base + 1024], ps[:, 0:1024])

            # Hidden layers
            prev = act
            for l in range(L):
                cur = actp.tile([128, BC // 4], fmm, tag="act")
                for u in range(4):
                    ps = psump.tile([128, 1024], f32, tag="ps")
                    for j in range(2):
                        # blockdiag(W_l.T x4): one MM does 4 strips' chunks
                        col = 1024 * u + 512 * j
                        nc.tensor.matmul(
                            ps[:, 512 * j : 512 * j + 512],
                            w4_sb[:, 128 * l : 128 * l + 128],
                            prev[:, col : col + 512],
                            start=True,
                            stop=True,
                        )
                    base = 1024 * u
                    if u % 2 == 0:
                        nc.scalar.activation(
                            cur[:, base : base + 1024],
                            ps[:, 0:1024],
                            mybir.ActivationFunctionType.Relu,
                            bias=b4_sb[:, l : l + 1],
                        )
                    else:
                        nc.vector.tensor_scalar(
                            cur[:, base : base + 1024],
                            ps[:, 0:1024],
                            b4_sb[:, l : l + 1],
                            0.0,
                            mybir.AluOpType.add,
                            mybir.AluOpType.max,
                        )
                prev = cur

            # Final layer, blockdiag: MM s writes y of chunk (t, s) to
            # psum partition 32t + s; 8 accumulating MMs share one bank.
            fin_ps = psump.tile([128, 1024], f32, tag="ps", name="fin_ps")
            for s in range(NSLICE):
                nc.tensor.matmul(
                    fin_ps[:, 0:512],
                    wfin_sb[:, 128 * s : 128 * s + 128],
                    prev[:, 512 * s : 512 * s + 512],
                    start=(s == 0),
                    stop=(s == NSLICE - 1),
                )
            # fin_ps[32t + s, 128c + p] = y[128p + 16s + 4c + t] (no bias yet).
            # Copy to SBUF, PE-transpose back to batch-major, add bias with a
            # shuffled output AP, then one contiguous DMA out.
            fin_sb = finp.tile([128, 512], f32, tag="fin")
            nc.scalar.activation(
                fin_sb, fin_ps[:, 0:512], mybir.ActivationFunctionType.Copy
            )
            tr_ps = psump.tile([128, 1024], f32, tag="ps", name="tr_ps")
            for c in range(4):
                nc.tensor.transpose(
                    tr_ps[:, 128 * c : 128 * c + 128],
                    fin_sb[:, 128 * c : 128 * c + 128],
                    ident_sb.bitcast(f32),
                )
            # tr_ps[p, 128c + 32t + s] -> fin2[p, 16s + 4c + t]
            fin2 = finp.tile([128, 128], f32, tag="fin2")
            nc.scalar.activation(
                fin2.rearrange("p (s c t) -> p c t s", s=8, c=4, t=4),
                tr_ps[:, 0:512].rearrange("p (c t s) -> p c t s", c=4, t=4, s=32)[
                    :, :, :, 0:8
                ],
                mybir.ActivationFunctionType.Identity,
                bias=bout_sb,
            )
            nc.sync.dma_start(
                out=y_d.rearrange("(p q) o -> p (q o)", p=128), in_=fin2
            )

    nc.compile()
    return nc


def _prep_weights(W_hidden, b_hidden, W_out, b_out):
    W_hidden = np.asarray(W_hidden, dtype=np.float32)
    b_hidden = np.asarray(b_hidden, dtype=np.float32)
    W_out = np.asarray(W_out, dtype=np.float32)
    b_out = np.asarray(b_out, dtype=np.float32)

    # w4[32r + i, 32l + o] = W_hidden[l, o, i]  (compact WT, replicated x4;
    # the kernel zero-fills and scatters this into the block-diagonal form)
    w4 = np.tile(W_hidden.transpose(2, 0, 1).reshape(32, L * 32), (4, 1)).copy()
    # b4[32c + o, l] = b_hidden[l, o]
    b4 = np.tile(b_hidden.T, (4, 1)).copy()
    # wfin[32t + i, 128s + m] = W_out[0, i] iff m == 32t + s  (s = 0..7)
    wfin = np.zeros((128, 1024), dtype=np.float32)
    for t in range(NSTRIP):
        for s in range(NSLICE):
            wfin[32 * t : 32 * t + 32, 128 * s + 32 * t + s] = W_out[0, :]
    bout = np.full((128, 1), b_out[0], dtype=np.float32)
    ident = np.eye(128, dtype=np.float32)
    return w4, b4, wfin, bout, ident


def kernel(state, W_hidden, b_hidden, W_out, b_out):
    from concourse import bass_utils

    state = np.asarray(state, dtype=np.float32)
    w4, b4, wfin, bout, ident = _prep_weights(W_hidden, b_hidden, W_out, b_out)

    if "nc" not in _CACHE:
        _CACHE["nc"] = _build_nc()
    nc = _CACHE["nc"]

    in_maps = []
    for i in range(NCORES):
        in_maps.append(
            {
                "x": state[BC * i : BC * (i + 1)].copy(),
                "w4": w4,
                "b4": b4,
                "wfin": wfin,
                "bout": bout,
                "ident": ident,
            }
        )
    res = bass_utils.run_bass_kernel_spmd(nc, in_maps, core_ids=list(range(NCORES)))
    y = np.concatenate([res.results[i]["y"] for i in range(NCORES)], axis=0)
    return y.astype(np.float32)


# revision 70
# speedup vs baseline: 1.0079x; 1.0012x over previous
"""Trainium2 Bass kernel for nn_Policy (40-layer 32-wide MLP over B=131072).

Strategy (pure data parallel over 8 cores, 16384 rows/core):
  - Activations live TRANSPOSED in SBUF: [32 features x batch] replicated in
    4 partition strips (strip t = partitions 32t..32t+32), 8 x 512-col slices
    per strip -> 32 chunks of 512 batch rows per core.
  - Each hidden layer = 2 rounds of 16 concurrent 32x32 tile-packed matmuls
    (tile_position=(32r, 32c)): weights stationary, activations moving.
    Tile (r, c) reads strip r slice 4k+c, writes PSUM bank r partitions 32c.
  - Fused bias+ReLU PSUM->SBUF, split between ScalarE (activation Relu with
    per-partition bias) and VectorE (dual-op tensor_scalar add/max).
  - Input transpose: contiguous DMA loads then PE transpose-mode 128x128.
  - Final layer: per chunk a [32,32] stationary with W_out in column s so 8
    chunks accumulate into distinct rows of one PSUM bank; one ScalarE copy
    (+b_out) then strided DMA out.
"""

import numpy as np

BC = 16384  # batch rows per core
D = 32
L = 40
NCORES = 8
NSTRIP = 4   # partition strips
NSLICE = 8   # 512-wide slices per strip
CHUNK = 512
SPLIT = 1104  # free-dim split point: ScalarE does [0:SPLIT], VectorE the rest
USE_F32R = True  # single-pass fp32 matmuls (4x PE throughput, ~tf32 precision)
LDW_OPT = True  # let walrus dedupe identical consecutive LDWEIGHTS

_CACHE = {}


def _patch_ldw_opt():
    if not LDW_OPT or _CACHE.get("ldw_patched"):
        return
    import concourse.bass_utils as bu

    orig = bu.run_command

    def patched(argv, **kw):
        argv = [
            "--enable-ldw-opt=true" if a == "--enable-ldw-opt=false" else a
            for a in argv
        ]
        return orig(argv, **kw)

    bu.run_command = patched
    _CACHE["ldw_patched"] = True


def _build_nc():
    import concourse.bacc as bacc
    import concourse.tile as tile
    from concourse import mybir

    _patch_ldw_opt()

    f32 = mybir.dt.float32
    fmm = mybir.dt.float32r if USE_F32R else f32
    nc = bacc.Bacc(
        "TRN2", target_bir_lowering=False, debug=False, enable_asserts=False
    )

    x_d = nc.dram_tensor("x", [BC, D], fmm, kind="ExternalInput").ap()
    w4_d = nc.dram_tensor("w4", [128, L * 32], fmm, kind="ExternalInput").ap()
    b4_d = nc.dram_tensor("b4", [128, L], f32, kind="ExternalInput").ap()
    wfin_d = nc.dram_tensor("wfin", [128, 1024], fmm, kind="ExternalInput").ap()
    bout_d = nc.dram_tensor("bout", [128, 1], f32, kind="ExternalInput").ap()
    ident_d = nc.dram_tensor("ident", [128, 128], fmm, kind="ExternalInput").ap()
    y_d = nc.dram_tensor("y", [BC, 1], f32, kind="ExternalOutput").ap()

    def mmcast(ap):
        return ap

    with tile.TileContext(nc) as tc:
        with (
            tc.tile_pool(name="consts", bufs=1) as consts,
            tc.tile_pool(name="xin", bufs=1) as xinp,
            tc.tile_pool(name="acts", bufs=4) as actp,
            tc.tile_pool(name="fin", bufs=1) as finp,
            tc.tile_pool(name="psum", bufs=4, space="PSUM") as psump,
        ):
            w4_sb = consts.tile([128, L * 128], fmm, tag="w4")
            b4_sb = consts.tile([128, L], f32, tag="b4")
            wfin_sb = consts.tile([128, 1024], fmm, tag="wfin")
            bout_sb = consts.tile([128, 1], f32, tag="bout")
            ident_sb = consts.tile([128, 128], fmm, tag="ident")
            nc.gpsimd.memset(w4_sb.bitcast(mybir.dt.uint32), 0)
            nc.sync.dma_start(out=ident_sb, in_=ident_d)

            # Input staging, fully contiguous: xin[p, 32u + i] = x[128p + u, i]
            # (16KB contiguous per partition -> full DMA bandwidth). The
            # resulting batch permutation is undone in the output DMA.
            xin_sb = xinp.tile([128, BC // 4], fmm, tag="xin")
            x_c = x_d.rearrange("(p u) i -> p (u i)", p=128)
            for g in range(8):
                eng = nc.sync if g % 2 == 0 else nc.scalar
                eng.dma_start(
                    out=xin_sb[:, 512 * g : 512 * (g + 1)],
                    in_=x_c[:, 512 * g : 512 * (g + 1)],
                )

            # Block-diagonal weights: zero-fill (above) then 4 strided DMAs
            # of the compact [128, L*32] replicated weights into the blocks.
            w4bd = w4_sb.rearrange("p (l m) -> p l m", m=128)
            for r in range(NSTRIP):
                nc.scalar.dma_start(
                    out=w4bd[32 * r : 32 * r + 32, :, 32 * r : 32 * r + 32],
                    in_=w4_d[32 * r : 32 * r + 32, :].rearrange(
                        "i (l o) -> i l o", o=32
                    ),
                )
            nc.gpsimd.dma_start(out=b4_sb, in_=b4_d)
            nc.gpsimd.dma_start(out=bout_sb, in_=bout_d)
            nc.gpsimd.dma_start(out=wfin_sb, in_=wfin_d)

            # PE warm-up during the input DMA: dummy matmuls on the identity
            # keep the HAM busy so transposes + early layers run at 2.4 GHz.
            wu_ps = psump.tile([128, 1024], f32, tag="ps", name="wu_ps")
            for k in range(32):
                nc.tensor.matmul(
                    wu_ps[:, 0:128],
                    ident_sb,
                    ident_sb,
                    start=True,
                    stop=True,
                    skip_group_check=True,
                )

            # Transpose into strip layout:
            # act[32t + i, 128n + p] = x[512n + 128t + p, i]
            act = actp.tile([128, BC // 4], fmm, tag="act")
            for g in range(4):
                ps = psump.tile([128, 1024], fmm, tag="ps")
                for j in range(8):
                    n = 8 * g + j
                    nc.tensor.transpose(
                        ps[:, 128 * j : 128 * j + 128],
                        xin_sb[:, 128 * n : 128 * n + 128],
                        ident_sb,
                    )
                base = 1024 * g
                if g % 2 == 0:
                    nc.scalar.activation(
                        act[:, base : base + 1024],
                        ps[:, 0:1024],
                        mybir.ActivationFunctionType.Copy,
                    )
                else:
                    nc.vector.tensor_copy(act[:, base : base + 1024], ps[:, 0:1024])

            # Hidden layers
            prev = act
            for l in range(L):
                cur = actp.tile([128, BC // 4], fmm, tag="act")
                for u in range(4):
                    ps = psump.tile([128, 1024], f32, tag="ps")
                    for j in range(2):
                        # blockdiag(W_l.T x4): one MM does 4 strips' chunks
                        col = 1024 * u + 512 * j
                        nc.tensor.matmul(
                            ps[:, 512 * j : 512 * j + 512],
                            w4_sb[:, 128 * l : 128 * l + 128],
                            prev[:, col : col + 512],
                            start=True,
                            stop=True,
                        )
                    base = 1024 * u
                    if u % 2 == 0:
                        nc.scalar.activation(
                            cur[:, base : base + 1024],
                            ps[:, 0:1024],
                            mybir.ActivationFunctionType.Relu,
                            bias=b4_sb[:, l : l + 1],
                        )
                    else:
                        nc.vector.tensor_scalar(
                            cur[:, base : base + 1024],
                            ps[:, 0:1024],
                            b4_sb[:, l : l + 1],
                            0.0,
                            mybir.AluOpType.add,
                            mybir.AluOpType.max,
                        )
                prev = cur

            # Final layer, blockdiag: MM s writes y of chunk (t, s) to
            # psum partition 32t + s; 8 accumulating MMs share one bank.
            fin_ps = psump.tile([128, 1024], f32, tag="ps", name="fin_ps")
            for s in range(NSLICE):
                nc.tensor.matmul(
                    fin_ps[:, 0:512],
                    wfin_sb[:, 128 * s : 128 * s + 128],
                    prev[:, 512 * s : 512 * s + 512],
                    start=(s == 0),
                    stop=(s == NSLICE - 1),
                )
            # fin_ps[32t + s, 128c + p] = y[128p + 16s + 4c + t] (no bias yet).
            # Copy to SBUF, PE-transpose back to batch-major, add bias with a
            # shuffled output AP, then one contiguous DMA out.
            fin_sb = finp.tile([128, 512], f32, tag="fin")
            nc.scalar.activation(
                fin_sb, fin_ps[:, 0:512], mybir.ActivationFunctionType.Copy
            )
            tr_ps = psump.tile([128, 1024], f32, tag="ps", name="tr_ps")
            for c in range(4):
                nc.tensor.transpose(
                    tr_ps[:, 128 * c : 128 * c + 128],
                    fin_sb[:, 128 * c : 128 * c + 128],
                    ident_sb.bitcast(f32),
                )
            # tr_ps[p, 128c + 32t + s] -> fin2[p, 16s + 4c + t]
            fin2 = finp.tile([128, 128], f32, tag="fin2")
            nc.scalar.activation(
                fin2.rearrange("p (s c t) -> p c t s", s=8, c=4, t=4),
                tr_ps[:, 0:512].rearrange("p (c t s) -> p c t s", c=4, t=4, s=32)[
                    :, :, :, 0:8
                ],
                mybir.ActivationFunctionType.Identity,
                bias=bout_sb,
            )
            nc.sync.dma_start(
                out=y_d.rearrange("(p q) o -> p (q o)", p=128), in_=fin2
            )

    nc.compile()
    return nc


def _prep_weights(W_hidden, b_hidden, W_out, b_out):
    W_hidden = np.asarray(W_hidden, dtype=np.float32)
    b_hidden = np.asarray(b_hidden, dtype=np.float32)
    W_out = np.asarray(W_out, dtype=np.float32)
    b_out = np.asarray(b_out, dtype=np.float32)

    # w4[32r + i, 32l + o] = W_hidden[l, o, i]  (compact WT, replicated x4;
    # the kernel zero-fills and scatters this into the block-diagonal form)
    w4 = np.tile(W_hidden.transpose(2, 0, 1).reshape(32, L * 32), (4, 1)).copy()
    # b4[32c + o, l] = b_hidden[l, o]
    b4 = np.tile(b_hidden.T, (4, 1)).copy()
    # wfin[32t + i, 128s + m] = W_out[0, i] iff m == 32t + s  (s = 0..7)
    wfin = np.zeros((128, 1024), dtype=np.float32)
    for t in range(NSTRIP):
        for s in range(NSLICE):
            wfin[32 * t : 32 * t + 32, 128 * s + 32 * t + s] = W_out[0, :]
    bout = np.full((128, 1), b_out[0], dtype=np.float32)
    ident = np.eye(128, dtype=np.float32)
    return w4, b4, wfin, bout, ident


def kernel(state, W_hidden, b_hidden, W_out, b_out):
    from concourse import bass_utils

    state = np.asarray(state, dtype=np.float32)
    w4, b4, wfin, bout, ident = _prep_weights(W_hidden, b_hidden, W_out, b_out)

    if "nc" not in _CACHE:
        _CACHE["nc"] = _build_nc()
    nc = _CACHE["nc"]

    in_maps = []
    for i in range(NCORES):
        in_maps.append(
            {
                "x": state[BC * i : BC * (i + 1)].copy(),
                "w4": w4,
                "b4": b4,
                "wfin": wfin,
                "bout": bout,
                "ident": ident,
            }
        )
    res = bass_utils.run_bass_kernel_spmd(nc, in_maps, core_ids=list(range(NCORES)))
    y = np.concatenate([res.results[i]["y"] for i in range(NCORES)], axis=0)
    return y.astype(np.float32)


# revision 71
# speedup vs baseline: 1.0231x; 1.0150x over previous
"""Trainium2 Bass kernel for nn_Policy (40-layer 32-wide MLP over B=131072).

Strategy (pure data parallel over 8 cores, 16384 rows/core):
  - Activations live TRANSPOSED in SBUF as [128 partitions x 4096]:
    partition 32t+i = feature i of strip t; each strip holds 8 chunks of
    512 batch columns. All matmul/activation steps preserve this layout.
  - Weights are float32r (single-pass fp32 matmul, 1 col/cycle vs 4 for
    full fp32) and BLOCK-DIAGONAL diag(W_l.T x4) [128,128], so ONE matmul
    [K=128, M=128, N=512] applies layer l to 4 strips' chunks at once:
    8 matmuls + 1 (walrus-deduped) LDWEIGHTS per layer.
  - Fused bias+ReLU PSUM->SBUF in four [128,1024] single-reader units per
    layer, alternating ScalarE activation(Relu, bias) / VectorE dual-op
    tensor_scalar(add,max) - both engines run saturated at ~2.4us/layer,
    which is the steady-state wall.
  - Input: contiguous DMA (16KB/partition) split over both HWDGE queues,
    then 32 PE transpose-mode 128x128 ops into the strip layout. Identity
    warm-up matmuls keep the PE HAM at full clock during the load.
  - Final layer: per slice s a block-diagonal stationary with W_out in
    column 32t+s accumulates all 32 chunks' outputs into one PSUM bank;
    PE-transpose back to batch-major, ScalarE adds b_out while undoing
    the layout permutation in its output AP, one contiguous DMA out.
"""

import numpy as np

BC = 16384  # batch rows per core
D = 32
L = 40
NCORES = 8
NSTRIP = 4   # partition strips
NSLICE = 8   # 512-wide slices per strip
CHUNK = 512
SPLIT = 1104  # free-dim split point: ScalarE does [0:SPLIT], VectorE the rest
USE_F32R = True  # single-pass fp32 matmuls (4x PE throughput, ~tf32 precision)
LDW_OPT = True  # let walrus dedupe identical consecutive LDWEIGHTS

_CACHE = {}


def _patch_ldw_opt():
    if not LDW_OPT or _CACHE.get("ldw_patched"):
        return
    import concourse.bass_utils as bu

    orig = bu.run_command

    def patched(argv, **kw):
        argv = [
            "--enable-ldw-opt=true" if a == "--enable-ldw-opt=false" else a
            for a in argv
        ]
        return orig(argv, **kw)

    bu.run_command = patched
    _CACHE["ldw_patched"] = True


def _build_nc():
    import concourse.bacc as bacc
    import concourse.tile as tile
    from concourse import mybir

    _patch_ldw_opt()

    f32 = mybir.dt.float32
    fmm = mybir.dt.float32r if USE_F32R else f32
    nc = bacc.Bacc(
        "TRN2", target_bir_lowering=False, debug=False, enable_asserts=False
    )

    x_d = nc.dram_tensor("x", [BC, D], fmm, kind="ExternalInput").ap()
    w4_d = nc.dram_tensor("w4", [128, L * 32], fmm, kind="ExternalInput").ap()
    b4_d = nc.dram_tensor("b4", [128, L], f32, kind="ExternalInput").ap()
    wfin_d = nc.dram_tensor("wfin", [128, 1024], fmm, kind="ExternalInput").ap()
    bout_d = nc.dram_tensor("bout", [128, 1], f32, kind="ExternalInput").ap()
    ident_d = nc.dram_tensor("ident", [128, 128], fmm, kind="ExternalInput").ap()
    y_d = nc.dram_tensor("y", [BC, 1], f32, kind="ExternalOutput").ap()

    def mmcast(ap):
        return ap

    with tile.TileContext(nc) as tc:
        with (
            tc.tile_pool(name="consts", bufs=1) as consts,
            tc.tile_pool(name="xin", bufs=1) as xinp,
            tc.tile_pool(name="acts", bufs=4) as actp,
            tc.tile_pool(name="fin", bufs=1) as finp,
            tc.tile_pool(name="psum", bufs=4, space="PSUM") as psump,
        ):
            w4_sb = consts.tile([128, L * 128], fmm, tag="w4")
            b4_sb = consts.tile([128, L], f32, tag="b4")
            wfin_sb = consts.tile([128, 1024], fmm, tag="wfin")
            bout_sb = consts.tile([128, 1], f32, tag="bout")
            ident_sb = consts.tile([128, 128], fmm, tag="ident")
            nc.gpsimd.memset(w4_sb.bitcast(mybir.dt.uint32), 0)
            nc.sync.dma_start(out=ident_sb, in_=ident_d)

            # Input staging, fully contiguous: xin[p, 32u + i] = x[128p + u, i]
            # (16KB contiguous per partition -> full DMA bandwidth). The
            # resulting batch permutation is undone in the output DMA.
            xin_sb = xinp.tile([128, BC // 4], fmm, tag="xin")
            x_c = x_d.rearrange("(p u) i -> p (u i)", p=128)
            for g in range(8):
                eng = nc.sync if g % 2 == 0 else nc.scalar
                eng.dma_start(
                    out=xin_sb[:, 512 * g : 512 * (g + 1)],
                    in_=x_c[:, 512 * g : 512 * (g + 1)],
                )

            # Block-diagonal weights: zero-fill (above) then 4 strided DMAs
            # of the compact [128, L*32] replicated weights into the blocks.
            w4bd = w4_sb.rearrange("p (l m) -> p l m", m=128)
            for r in range(NSTRIP):
                nc.scalar.dma_start(
                    out=w4bd[32 * r : 32 * r + 32, :, 32 * r : 32 * r + 32],
                    in_=w4_d[32 * r : 32 * r + 32, :].rearrange(
                        "i (l o) -> i l o", o=32
                    ),
                )
            nc.gpsimd.dma_start(out=b4_sb, in_=b4_d)
            nc.gpsimd.dma_start(out=bout_sb, in_=bout_d)
            nc.gpsimd.dma_start(out=wfin_sb, in_=wfin_d)

            # PE warm-up during the input DMA: dummy matmuls on the identity
            # keep the HAM busy so transposes + early layers run at 2.4 GHz.
            wu_ps = psump.tile([128, 1024], f32, tag="ps", name="wu_ps")
            for k in range(32):
                nc.tensor.matmul(
                    wu_ps[:, 0:128],
                    ident_sb,
                    ident_sb,
                    start=True,
                    stop=True,
                    skip_group_check=True,
                )

            # Transpose into strip layout:
            # act[32t + i, 128n + p] = x[512n + 128t + p, i]
            act = actp.tile([128, BC // 4], fmm, tag="act")
            for g in range(4):
                ps = psump.tile([128, 1024], fmm, tag="ps")
                for j in range(8):
                    n = 8 * g + j
                    nc.tensor.transpose(
                        ps[:, 128 * j : 128 * j + 128],
                        xin_sb[:, 128 * n : 128 * n + 128],
                        ident_sb,
                    )
                base = 1024 * g
                if g % 2 == 0:
                    nc.scalar.activation(
                        act[:, base : base + 1024],
                        ps[:, 0:1024],
                        mybir.ActivationFunctionType.Copy,
                    )
                else:
                    nc.vector.tensor_copy(act[:, base : base + 1024], ps[:, 0:1024])

            # Hidden layers
            prev = act
            for l in range(L):
                cur = actp.tile([128, BC // 4], fmm, tag="act")
                for u in range(4):
                    ps = psump.tile([128, 1024], f32, tag="ps")
                    for j in range(2):
                        # blockdiag(W_l.T x4): one MM does 4 strips' chunks
                        col = 1024 * u + 512 * j
                        nc.tensor.matmul(
                            ps[:, 512 * j : 512 * j + 512],
                            w4_sb[:, 128 * l : 128 * l + 128],
                            prev[:, col : col + 512],
                            start=True,
                            stop=True,
                        )
                    base = 1024 * u
                    if u % 2 == 0:
                        nc.scalar.activation(
                            cur[:, base : base + 1024],
                            ps[:, 0:1024],
                            mybir.ActivationFunctionType.Relu,
                            bias=b4_sb[:, l : l + 1],
                        )
                    else:
                        nc.vector.tensor_scalar(
                            cur[:, base : base + 1024],
                            ps[:, 0:1024],
                            b4_sb[:, l : l + 1],
                            0.0,
                            mybir.AluOpType.add,
                            mybir.AluOpType.max,
                        )
                prev = cur

            # Final layer, blockdiag: MM s writes y of chunk (t, s) to
            # psum partition 32t + s; 8 accumulating MMs share one bank.
            fin_ps = psump.tile([128, 1024], f32, tag="ps", name="fin_ps")
            for s in range(NSLICE):
                nc.tensor.matmul(
                    fin_ps[:, 0:512],
                    wfin_sb[:, 128 * s : 128 * s + 128],
                    prev[:, 512 * s : 512 * s + 512],
                    start=(s == 0),
                    stop=(s == NSLICE - 1),
                )
            # fin_ps[32t + s, 128c + p] = y[128p + 16s + 4c + t] (no bias yet).
            # Copy to SBUF, PE-transpose back to batch-major, add bias with a
            # shuffled output AP, then one contiguous DMA out.
            fin_sb = finp.tile([128, 512], f32, tag="fin")
            nc.scalar.activation(
                fin_sb, fin_ps[:, 0:512], mybir.ActivationFunctionType.Copy
            )
            tr_ps = psump.tile([128, 1024], f32, tag="ps", name="tr_ps")
            for c in range(4):
                nc.tensor.transpose(
                    tr_ps[:, 128 * c : 128 * c + 128],
                    fin_sb[:, 128 * c : 128 * c + 128],
                    ident_sb.bitcast(f32),
                )
            # tr_ps[p, 128c + 32t + s] -> fin2[p, 16s + 4c + t]
            fin2 = finp.tile([128, 128], f32, tag="fin2")
            nc.scalar.activation(
                fin2.rearrange("p (s c t) -> p c t s", s=8, c=4, t=4),
                tr_ps[:, 0:512].rearrange("p (c t s) -> p c t s", c=4, t=4, s=32)[
                    :, :, :, 0:8
                ],
                mybir.ActivationFunctionType.Identity,
                bias=bout_sb,
            )
            nc.sync.dma_start(
                out=y_d.rearrange("(p q) o -> p (q o)", p=128), in_=fin2
            )

    nc.compile()
    return nc


def _prep_weights(W_hidden, b_hidden, W_out, b_out):
    W_hidden = np.asarray(W_hidden, dtype=np.float32)
    b_hidden = np.asarray(b_hidden, dtype=np.float32)
    W_out = np.asarray(W_out, dtype=np.float32)
    b_out = np.asarray(b_out, dtype=np.float32)

    # w4[32r + i, 32l + o] = W_hidden[l, o, i]  (compact WT, replicated x4;
    # the kernel zero-fills and scatters this into the block-diagonal form)
    w4 = np.tile(W_hidden.transpose(2, 0, 1).reshape(32, L * 32), (4, 1)).copy()
    # b4[32c + o, l] = b_hidden[l, o]
    b4 = np.tile(b_hidden.T, (4, 1)).copy()
    # wfin[32t + i, 128s + m] = W_out[0, i] iff m == 32t + s  (s = 0..7)
    wfin = np.zeros((128, 1024), dtype=np.float32)
    for t in range(NSTRIP):
        for s in range(NSLICE):
            wfin[32 * t : 32 * t + 32, 128 * s + 32 * t + s] = W_out[0, :]
    bout = np.full((128, 1), b_out[0], dtype=np.float32)
    ident = np.eye(128, dtype=np.float32)
    return w4, b4, wfin, bout, ident


def kernel(state, W_hidden, b_hidden, W_out, b_out):
    from concourse import bass_utils

    state = np.asarray(state, dtype=np.float32)
    w4, b4, wfin, bout, ident = _prep_weights(W_hidden, b_hidden, W_out, b_out)

    if "nc" not in _CACHE:
        _CACHE["nc"] = _build_nc()
    nc = _CACHE["nc"]

    in_maps = []
    for i in range(NCORES):
        in_maps.append(
            {
                "x": state[BC * i : BC * (i + 1)].copy(),
                "w4": w4,
                "b4": b4,
                "wfin": wfin,
                "bout": bout,
                "ident": ident,
            }
        )
    res = bass_utils.run_bass_kernel_spmd(nc, in_maps, core_ids=list(range(NCORES)))
    y = np.concatenate([res.results[i]["y"] for i in range(NCORES)], axis=0)
    return y.astype(np.float32)
